# revision 85
# baseline (speedup 1.0000x reference)
"""Trainium2 Bass kernel for a SimCLR-style contrastive loss (v4).

Math (per batch item b, xn/yn L2-normalized rows, tau = 0.01):
  x-row i logits = {S_xy[i, :]} u {S_xx[i, j != i]}    (2n-1 values)
  y-row j logits = {S_xy[:, j]} u {S_yy[j, i != j]}
  loss = mean over bs*2n rows of (logsumexp(logits) - S_xy[diag])

Per-core structure (data-parallel over bs, 2 items/core). Key design
points, driven by perfetto traces of earlier versions:

  - ONE strided DMA per tensor loads [n, d] into natural SBUF layout.
  - ssq via fused DVE scalar_tensor_tensor (accum_out); 10/||row|| =
    exp(-.5*ln(ss)+ln10) on ScalarE.  An explicit InstLoadActFuncSet pins
    the natural_log_exp table set so the whole kernel does ONE activation
    table load (the default placement thrashed exp/ln sets 7 times).
  - rows scaled+cast to bf16 by DVE tensor_scalar (fp32 2x mode).
  - operand transposes via the DMA XBAR (store scaled bf16 to DRAM, read
    back with dma_start_transpose): zero PE/DVE cost.
  - phase order XX -> XY -> YY so the PE can start as soon as x alone is
    prepped (~20us earlier than an XY-first order).
  - S_xy: full matrix; rowsums from ACT exp accum_out, colsums via
    ones-vector PE matmuls accumulated in PSUM.
  - S_xx/S_yy: upper triangle only; diagonal masked by a -1e5*identity
    matmul folded into the PSUM accumulation; row totals = ACT rowsum of
    the [lo, n) strip + colsums of the strictly-upper blocks (symmetry),
    moved to [128, 8] layout via a [1,n] DRAM roundtrip per phase.
  - pos via fused DVE mul-reduce, emitted after the matmul phases (off
    the critical path); ln on ACT with accum; one ones-matmul collapses
    partitions; host sums the 8 per-core partials.
"""

from contextlib import ExitStack

import numpy as np

import concourse.bacc as bacc
import concourse.tile as tile
from concourse import mybir
from concourse.bass_utils import run_bass_kernel_spmd

BS, N, D = 16, 1024, 256
NCORES = 8
IPC = BS // NCORES  # items per core
P = 128
NT = N // P  # 128-row blocks per item
KC = D // P  # contraction chunks
HB = 512  # one PSUM bank of fp32
NEG = -100000.0  # folded into S_xx/S_yy diag -> exp() == 0.0
LN10 = 2.302585092994046

dt = mybir.dt
AF = mybir.ActivationFunctionType
ALU = mybir.AluOpType
AX = mybir.AxisListType
F32 = dt.float32
BF16 = dt.bfloat16


def _pin_act_table(nc):
    """Emit an explicit table load for the set containing BOTH Exp and Ln,
    so bacc's fixpoint pass never needs to swap tables mid-kernel."""
    from concourse.hw_specs import get_activation_tables

    tabs = list(get_activation_tables(nc.m.arch).items())
    setid = next(i for i, (_, fns) in enumerate(tabs)
                 if AF.Exp in fns and AF.Ln in fns)
    nc.scalar.add_instruction(mybir.InstLoadActFuncSet(
        name=nc.get_next_instruction_name(), ins=[], outs=[],
        act_func_set_id=setid))


def build_nc():
    nc = bacc.Bacc("TRN2", target_bir_lowering=False, debug=False)

    x_in = nc.dram_tensor("x", [IPC * N, D], F32, kind="ExternalInput")
    y_in = nc.dram_tensor("y", [IPC * N, D], F32, kind="ExternalInput")
    idt_in = nc.dram_tensor("idt", [P, P], BF16, kind="ExternalInput")
    negid_in = nc.dram_tensor("negid", [P, P], BF16, kind="ExternalInput")
    out_d = nc.dram_tensor("out", [P, 3 * IPC], F32, kind="ExternalOutput")

    with tile.TileContext(nc) as tc, ExitStack() as ctx:
        const = ctx.enter_context(tc.tile_pool(name="const", bufs=1))
        nat = ctx.enter_context(tc.tile_pool(name="nat", bufs=2))
        xbp = ctx.enter_context(tc.tile_pool(name="xbp", bufs=2))
        opT = ctx.enter_context(tc.tile_pool(name="opT", bufs=2))
        eep = ctx.enter_context(tc.tile_pool(name="eep", bufs=8))
        stat = ctx.enter_context(tc.tile_pool(name="stat", bufs=2))
        scr = ctx.enter_context(tc.tile_pool(name="scr", bufs=3))
        cssb = ctx.enter_context(tc.tile_pool(name="cssb", bufs=3))
        finp = ctx.enter_context(tc.tile_pool(name="finp", bufs=1))
        dram = ctx.enter_context(tc.tile_pool(name="dram", bufs=2, space="DRAM"))
        ps2 = ctx.enter_context(tc.tile_pool(name="ps2", bufs=2, space="PSUM"))
        ps1 = ctx.enter_context(tc.tile_pool(name="ps1", bufs=2, space="PSUM"))
        psc = ctx.enter_context(tc.tile_pool(name="psc", bufs=2, space="PSUM"))

        _pin_act_table(nc)

        # input loads ride the gpsimd software-DGE queue: cheap to issue and
        # on separate DMA rings, so they never head-of-line-block the
        # store->XBAR-transpose chain on the HWDGE queue.
        def load_nat(tname, it, src):
            # two half-loads so the ssq chain starts on the first half while
            # the second is still in flight
            t = nat.tile([P, NT * D], F32, tag=f"nat{tname}", name=f"nat{tname}{it}")
            H = NT // 2
            for h in range(2):
                nc.gpsimd.dma_start(
                    t[:, h * H * D:(h + 1) * H * D].rearrange(
                        "p (m d) -> p m d", m=H),
                    src[it * N + h * H * P:it * N + (h + 1) * H * P, :]
                    .rearrange("(m p) d -> p m d", p=P))
            return t

        nx0 = load_nat("x", 0, x_in)
        ny0 = load_nat("y", 0, y_in)

        idt = const.tile([P, P], BF16, tag="idt")
        nc.sync.dma_start(idt[:], idt_in[:])
        negid = const.tile([P, P], BF16, tag="negid")
        nc.sync.dma_start(negid[:], negid_in[:])
        ones_b = const.tile([P, 1], BF16, tag="ones_b")
        nc.vector.memset(ones_b[:], 1.0)
        zeros_b = const.tile([P, 1], BF16, tag="zeros_b")
        nc.vector.memset(zeros_b[:], 0.0)
        ones_f = const.tile([P, 1], F32, tag="ones_f")
        nc.vector.memset(ones_f[:], 1.0)
        ln10c = const.tile([P, 1], F32, tag="ln10c")
        nc.vector.memset(ln10c[:], LN10)

        # fin columns per item: [sum ln Tx, sum ln Ty, -2*pos_sum]
        fin = finp.tile([P, 3 * IPC], F32, tag="fin")

        def prep_norm(tname, it, nt_, act_ssq=False):
            """fused ssq per row block (DVE, or ScalarE Square when the DVE
            is the gating engine), then 10/||row|| on ScalarE. Processed in
            halves so scaling can begin before the second half of the input
            DMA has landed."""
            H = NT // 2
            ss = stat.tile([P, NT], F32, tag=f"ss{tname}", name=f"ss{tname}{it}")
            inv10 = stat.tile([P, NT], F32, tag=f"inv{tname}", name=f"inv{tname}{it}")
            for h in range(2):
                for mt in range(h * H, (h + 1) * H):
                    sq = scr.tile([P, D], BF16, tag="sq", name="sq")
                    if act_ssq:
                        nc.scalar.activation(
                            sq[:], nt_[:, mt * D:(mt + 1) * D], AF.Square,
                            accum_out=ss[:, mt:mt + 1])
                    else:
                        nc.vector.scalar_tensor_tensor(
                            sq[:], nt_[:, mt * D:(mt + 1) * D], 1.0,
                            nt_[:, mt * D:(mt + 1) * D], ALU.mult, ALU.mult,
                            accum_out=ss[:, mt:mt + 1])
                lns = scr.tile([P, NT], F32, tag="lns", name="lns")
                nc.scalar.activation(lns[:, h * H:(h + 1) * H],
                                     ss[:, h * H:(h + 1) * H], AF.Ln)
                nc.scalar.activation(inv10[:, h * H:(h + 1) * H],
                                     lns[:, h * H:(h + 1) * H], AF.Exp,
                                     scale=-0.5, bias=ln10c[:])
            return inv10

        def prep_scale(tname, it, nt_, inv10, pe_transpose=False, hsplit=False):
            """scale+cast rows to bf16, then build the d-major operands."""
            b = xbp.tile([P, NT * D], BF16, tag=f"{tname}b", name=f"{tname}b{it}")
            for mt in range(NT):
                nc.vector.tensor_scalar(
                    b[:, mt * D:(mt + 1) * D], nt_[:, mt * D:(mt + 1) * D],
                    inv10[:, mt:mt + 1], None, ALU.mult)
            # operands are built as half-tiles ts[k][h] = [128 (d-chunk k),
            # 512 (rows h*512..)]: every matmul slice in the kernel aligns
            # with the 512-wide PSUM bank halves, and the DMA-transpose path
            # can ship each row-half as soon as its scaling lands.
            ts = [[None, None] for _ in range(KC)]
            H = NT // 2
            if pe_transpose:
                # PE transposes pipeline with the per-block scaling and skip
                # the DRAM roundtrip; drain copies run on ScalarE (idle here)
                # to keep the DVE free for the other operand's prep chain.
                for k in range(KC):
                    tp = ps1.tile([P, N], BF16, tag="ps1", name="tp")
                    for mt in range(NT):
                        nc.tensor.transpose(
                            tp[:, mt * P:(mt + 1) * P],
                            b[:, mt * D + k * P:mt * D + (k + 1) * P], idt[:])
                    for h in range(2):
                        tT = opT.tile([P, HB], BF16, tag=f"{tname}T{k}{h}",
                                      name=f"{tname}T{k}{h}_{it}")
                        nc.scalar.copy(tT[:], tp[:, h * HB:(h + 1) * HB])
                        ts[k][h] = tT
            elif hsplit:
                # per-row-half store + transpose: the first half ships while
                # the second is still being scaled. Costs 2x the Sync issue
                # slots, so only used where the operand latency is critical.
                bd = dram.tile([N, D], BF16, tag=f"{tname}bd", name=f"{tname}bd{it}")
                bv = b[:].rearrange("p (m d) -> p m d", m=NT)
                for h in range(2):
                    nc.sync.dma_start(
                        bd[h * HB:(h + 1) * HB, :].rearrange(
                            "(m p) d -> p m d", p=P),
                        bv[:, h * H:(h + 1) * H, :])
                    for k in range(KC):
                        tT = opT.tile([P, HB], BF16, tag=f"{tname}T{k}{h}",
                                      name=f"{tname}T{k}{h}_{it}")
                        nc.sync.dma_start_transpose(
                            tT[:], bd[h * HB:(h + 1) * HB, k * P:(k + 1) * P])
                        ts[k][h] = tT
            else:
                bd = dram.tile([N, D], BF16, tag=f"{tname}bd", name=f"{tname}bd{it}")
                nc.sync.dma_start(
                    bd[:].rearrange("(m p) d -> p m d", p=P),
                    b[:].rearrange("p (m d) -> p m d", m=NT))
                for k in range(KC):
                    tT = opT.tile([P, N], BF16, tag=f"{tname}T{k}",
                                  name=f"{tname}T{k}_{it}")
                    nc.sync.dma_start_transpose(tT[:], bd[:, k * P:(k + 1) * P])
                    for h in range(2):
                        ts[k][h] = tT[:, h * HB:(h + 1) * HB]
            return b, ts

        def roundtrip(vtag, it, drains):
            """PSUM [1, n] colsum vectors -> SBUF staging -> DRAM -> [128, 8]."""
            sb = cssb.tile([1, N], F32, tag="cs_sb", name=f"sb_{vtag}{it}")
            bcs = dram.tile([NT, P], F32, tag="bcs", name=f"bcs_{vtag}{it}")
            for (dst0, dst1, src) in drains:
                nc.vector.tensor_copy(sb[:, dst0:dst1], src)
            nc.sync.dma_start(bcs[:], sb[:])
            csT = stat.tile([P, NT], F32, tag=f"csT{vtag}", name=f"csT{vtag}{it}")
            nc.sync.dma_start(csT[:], bcs.rearrange("j p -> p j"))
            return csT

        def roundtrip_pe(vtag, it, drains):
            """Like roundtrip(), but the [1,n] -> [128,8] layout flip runs as
            8 tiny K=1 PE transposes into PSUM: ~3us less latency than the
            DRAM bounce. Used for the last phase, where it's tail-exposed
            and the PE is idle."""
            sb = cssb.tile([1, N], F32, tag="cs_sb", name=f"sb_{vtag}{it}")
            for (dst0, dst1, src) in drains:
                nc.vector.tensor_copy(sb[:, dst0:dst1], src)
            csp = psc.tile([P, NT], F32, tag="cs", name=f"csp_{vtag}{it}")
            for j in range(NT):
                nc.tensor.transpose(csp[:, j:j + 1], sb[0:1, j * P:(j + 1) * P],
                                    ones_f[0:1, 0:1])
            return csp

        # sym-phase row-block order: alternate between the ps1 pool (mt>=4,
        # one bank) and ps2 (mt<4, two banks) so up to 4 row-blocks are in
        # flight and the PE stream stays dense (keeps the HAM clock warm).
        SYM_ORDER = [4, 0, 5, 1, 6, 2, 3, 7]
        # last contributing mt per colsum half, in emission order
        _contrib = {nh: [mt for mt in SYM_ORDER
                         if max(mt * P + P, nh * HB) < min((nh + 1) * HB, N)]
                    for nh in range(2)}

        def sym_phase(oT, sname, it, bridge, warm=0, tail=False):
            """Upper-triangle similarity phase: returns (rowsums, csT)."""
            rs = stat.tile([P, NT], F32, tag=f"rs{sname}", name=f"rs{sname}{it}")
            cs = [psc.tile([1, HB], F32, tag="cs", name=f"cs{sname}{nh}_{it}")
                  for nh in range(2)]
            # open each accumulation group with a full-region zeroing matmul
            # so later partial-region contributors see uniform has_written
            # state (also makes cs[0][:, 0:128] valid zeros for the drain).
            # `bridge` (the scaled natural tile) is ready well before the
            # DMA-transposed operands, so openers + warm-up matmuls run in
            # the store->transpose latency window and bring the PE clock to
            # 2.4 GHz before the first real matmul.
            def emit_openers():
                for nh in range(2):
                    nc.tensor.matmul(cs[nh][:], zeros_b[:], bridge[:, 0:HB],
                                     start=True, stop=False)
                for w in range(warm):
                    nc.tensor.matmul(cs[w % 2][:], zeros_b[:],
                                     bridge[:, 0:HB], start=False, stop=False)

            first = True
            for mt in SYM_ORDER:
                lo = mt * P
                if lo < HB:
                    ps = ps2.tile([P, N], F32, tag="ps2", name="ps_sym")
                    base = 0
                    chunks = [(lo, HB), (HB, N)]
                else:
                    ps = ps1.tile([P, HB], F32, tag="ps1", name="ps_sym1")
                    base = HB
                    chunks = [(lo, N)]
                for ci, (c0, c1) in enumerate(chunks):
                    ch = c0 // HB
                    for k in range(KC):
                        nc.tensor.matmul(
                            ps[:, c0 - base:c1 - base],
                            oT[k][mt // 4][:, (mt % 4) * P:(mt % 4 + 1) * P],
                            oT[k][ch][:, c0 - ch * HB:c1 - ch * HB],
                            start=(k == 0),
                            stop=(k == KC - 1 and ci > 0))
                # diag mask: add -1e5*I to [lo, lo+P) inside the group
                nc.tensor.matmul(
                    ps[:, lo - base:lo - base + P], idt[:], negid[:],
                    start=False, stop=True)
                if first:
                    # openers sit after the first block's main matmuls so a
                    # phase can start computing before the previous phase's
                    # cs slots have drained
                    emit_openers()
                    first = False
                ee = eep.tile([P, N], BF16, tag="ee", name="ee_sym")
                nc.scalar.activation(ee[:, lo:], ps[:, lo - base:],
                                     AF.Exp, accum_out=rs[:, mt:mt + 1])
                # strictly-upper colsums (lower-triangle rowsums by symmetry)
                for nh in range(2):
                    a = max(lo + P, nh * HB)
                    b = min((nh + 1) * HB, N)
                    if a >= b:
                        continue
                    nc.tensor.matmul(
                        cs[nh][:, a - nh * HB:b - nh * HB],
                        ones_b[:], ee[:, a:b],
                        start=False,
                        stop=(mt == _contrib[nh][-1]))
            return rs, cs

        def xy_phase(it, xT, yT):
            rs_xy = stat.tile([P, NT], F32, tag="rs_xy", name=f"rs_xy{it}")
            cs_xy = [psc.tile([1, HB], F32, tag="cs", name=f"cs_xy{nh}_{it}")
                     for nh in range(2)]
            for mt in range(NT):
                ps = ps2.tile([P, N], F32, tag="ps2", name="ps_xy")
                for nh in range(2):
                    for k in range(KC):
                        nc.tensor.matmul(
                            ps[:, nh * HB:(nh + 1) * HB],
                            xT[k][mt // 4][:, (mt % 4) * P:(mt % 4 + 1) * P],
                            yT[k][nh][:],
                            start=(k == 0), stop=(k == KC - 1))
                ee = eep.tile([P, N], BF16, tag="ee", name="ee_xy")
                nc.scalar.activation(ee[:], ps[:], AF.Exp,
                                     accum_out=rs_xy[:, mt:mt + 1])
                for nh in range(2):
                    nc.tensor.matmul(
                        cs_xy[nh][:], ones_b[:], ee[:, nh * HB:(nh + 1) * HB],
                        start=(mt == 0), stop=(mt == NT - 1))
            return rs_xy, cs_xy

        def pos_diag(it, xb, yb):
            pos = stat.tile([P, NT], F32, tag="pos", name=f"pos{it}")
            for mt in range(NT):
                pq = scr.tile([P, D], BF16, tag="pq", name="pq")
                nc.vector.scalar_tensor_tensor(
                    pq[:], xb[:, mt * D:(mt + 1) * D], 1.0,
                    yb[:, mt * D:(mt + 1) * D], ALU.mult, ALU.mult,
                    accum_out=pos[:, mt:mt + 1])
            return pos

        _D = lambda cs: [(0, HB, cs[0][:]), (HB, N, cs[1][:])]

        # ---- software-pipelined emission: each engine's queue is FIFO, so
        # emission order IS the schedule. Norm (ScalarE) ops for the next
        # operand are emitted before a phase's exp stream; item1's prep is
        # interleaved between item0's phases.
        invx0 = prep_norm("x", 0, nx0)
        xb0, xT0 = prep_scale("x", 0, nx0, invx0, pe_transpose=True)
        nx1 = load_nat("x", 1, x_in)
        ny1 = load_nat("y", 1, y_in)
        invx1 = prep_norm("x", 1, nx1)
        xb1, xT1 = prep_scale("x", 1, nx1, invx1)

        rs_xx0, cs_xx0 = sym_phase(xT0, "xx", 0, xb0, warm=4)

        # y0's norm/scale emitted after XX0 so its ScalarE ops slot into the
        # exp stream without stalling it; XX1 (x-only) fills the window where
        # XY0 would otherwise wait on y0's store->transpose chain.
        invy0 = prep_norm("y", 0, ny0)
        yb0, yT0 = prep_scale("y", 0, ny0, invy0, hsplit=True)
        csT_xx0 = roundtrip("xx", 0, _D(cs_xx0))

        rs_xx1, cs_xx1 = sym_phase(xT1, "xx", 1, xb1)
        csT_xx1 = roundtrip("xx", 1, _D(cs_xx1))

        rs_xy0, cs_xy0 = xy_phase(0, xT0, yT0)

        invy1 = prep_norm("y", 1, ny1)
        yb1, yT1 = prep_scale("y", 1, ny1, invy1)
        csT_xy0 = roundtrip("xy", 0, _D(cs_xy0))

        def combine(it, rs_xx, csT_xx, rs_xy, csT_xy, rs_yy, csT_yy, pos):
            tx = stat.tile([P, NT], F32, tag="tx", name=f"tx{it}")
            nc.vector.tensor_add(tx[:], rs_xy[:], rs_xx[:])
            nc.vector.tensor_add(tx[:], tx[:], csT_xx[:])
            lnx = scr.tile([P, NT], F32, tag="lnx", name="lnx")
            nc.scalar.activation(lnx[:], tx[:], AF.Ln,
                                 accum_out=fin[:, 3 * it:3 * it + 1])

            ty = stat.tile([P, NT], F32, tag="ty", name=f"ty{it}")
            nc.vector.tensor_add(ty[:], rs_yy[:], csT_xy[:])
            nc.vector.tensor_add(ty[:], ty[:], csT_yy[:])
            lny = scr.tile([P, NT], F32, tag="lny", name="lny")
            nc.scalar.activation(lny[:], ty[:], AF.Ln,
                                 accum_out=fin[:, 3 * it + 1:3 * it + 2])

            posr = stat.tile([P, 1], F32, tag="posr", name=f"posr{it}")
            nc.vector.reduce_sum(posr[:], pos[:], axis=AX.X)
            nc.vector.tensor_scalar_mul(fin[:, 3 * it + 2:3 * it + 3], posr[:], -2.0)

        rs_yy0, cs_yy0 = sym_phase(yT0, "yy", 0, yb0)
        csT_yy0 = roundtrip("yy", 0, _D(cs_yy0))
        pos0 = pos_diag(0, xb0, yb0)
        combine(0, rs_xx0, csT_xx0, rs_xy0, csT_xy0, rs_yy0, csT_yy0, pos0)

        rs_xy1, cs_xy1 = xy_phase(1, xT1, yT1)
        csT_xy1 = roundtrip("xy", 1, _D(cs_xy1))

        rs_yy1, cs_yy1 = sym_phase(yT1, "yy", 1, yb1)
        csT_yy1 = roundtrip_pe("yy", 1, _D(cs_yy1))
        pos1 = pos_diag(1, xb1, yb1)
        combine(1, rs_xx1, csT_xx1, rs_xy1, csT_xy1, rs_yy1, csT_yy1, pos1)

        # ship the [128, 6] partials raw; the host does the final
        # reduction (drops the fin-matmul -> copy -> reduce tail chain)
        nc.sync.dma_start(out_d[:], fin[:])

    nc.compile()
    return nc


_CACHE = {}
TRACE = False
LAST_RESULTS = None


def _get_nc():
    if "nc" not in _CACHE:
        _CACHE["nc"] = build_nc()
    return _CACHE["nc"]


def make_in_maps(x, y):
    import ml_dtypes

    x = np.ascontiguousarray(np.asarray(x, dtype=np.float32))
    y = np.ascontiguousarray(np.asarray(y, dtype=np.float32))
    idt = np.eye(P, dtype=np.float32).astype(ml_dtypes.bfloat16)
    negid = (np.eye(P, dtype=np.float32) * NEG).astype(ml_dtypes.bfloat16)
    in_maps = []
    for c in range(NCORES):
        in_maps.append({
            "x": x[c * IPC:(c + 1) * IPC].reshape(IPC * N, D),
            "y": y[c * IPC:(c + 1) * IPC].reshape(IPC * N, D),
            "idt": idt,
            "negid": negid,
        })
    return in_maps


def kernel(x, y):
    global LAST_RESULTS
    nc = _get_nc()
    in_maps = make_in_maps(x, y)
    res = run_bass_kernel_spmd(nc, in_maps, list(range(NCORES)), trace=TRACE)
    LAST_RESULTS = res
    partials = np.array([np.asarray(r["out"], dtype=np.float64).sum()
                         for r in res.results])
    return np.float32(partials.sum() / (BS * 2 * N))


# revision 86
# speedup vs baseline: 1.0207x; 1.0207x over previous
"""Trainium2 Bass kernel for a SimCLR-style contrastive loss (v4).

Math (per batch item b, xn/yn L2-normalized rows, tau = 0.01):
  x-row i logits = {S_xy[i, :]} u {S_xx[i, j != i]}    (2n-1 values)
  y-row j logits = {S_xy[:, j]} u {S_yy[j, i != j]}
  loss = mean over bs*2n rows of (logsumexp(logits) - S_xy[diag])

Per-core structure (data-parallel over bs, 2 items/core). Key design
points, driven by perfetto traces of earlier versions:

  - ONE strided DMA per tensor loads [n, d] into natural SBUF layout.
  - ssq via fused DVE scalar_tensor_tensor (accum_out); 10/||row|| =
    exp(-.5*ln(ss)+ln10) on ScalarE.  An explicit InstLoadActFuncSet pins
    the natural_log_exp table set so the whole kernel does ONE activation
    table load (the default placement thrashed exp/ln sets 7 times).
  - rows scaled+cast to bf16 by DVE tensor_scalar (fp32 2x mode).
  - operand transposes via the DMA XBAR (store scaled bf16 to DRAM, read
    back with dma_start_transpose): zero PE/DVE cost.
  - phase order XX -> XY -> YY so the PE can start as soon as x alone is
    prepped (~20us earlier than an XY-first order).
  - S_xy: full matrix; rowsums from ACT exp accum_out, colsums via
    ones-vector PE matmuls accumulated in PSUM.
  - S_xx/S_yy: upper triangle only; diagonal masked by a -1e5*identity
    matmul folded into the PSUM accumulation; row totals = ACT rowsum of
    the [lo, n) strip + colsums of the strictly-upper blocks (symmetry),
    moved to [128, 8] layout via a [1,n] DRAM roundtrip per phase.
  - pos via fused DVE mul-reduce, emitted after the matmul phases (off
    the critical path); ln on ACT with accum; one ones-matmul collapses
    partitions; host sums the 8 per-core partials.
"""

from contextlib import ExitStack

import numpy as np

import concourse.bacc as bacc
import concourse.tile as tile
from concourse import mybir
from concourse.bass_utils import run_bass_kernel_spmd

BS, N, D = 16, 1024, 256
NCORES = 8
IPC = BS // NCORES  # items per core
P = 128
NT = N // P  # 128-row blocks per item
KC = D // P  # contraction chunks
HB = 512  # one PSUM bank of fp32
NEG = -100000.0  # folded into S_xx/S_yy diag -> exp() == 0.0
LN10 = 2.302585092994046

dt = mybir.dt
AF = mybir.ActivationFunctionType
ALU = mybir.AluOpType
AX = mybir.AxisListType
F32 = dt.float32
BF16 = dt.bfloat16


def _pin_act_table(nc):
    """Emit an explicit table load for the set containing BOTH Exp and Ln,
    so bacc's fixpoint pass never needs to swap tables mid-kernel."""
    from concourse.hw_specs import get_activation_tables

    tabs = list(get_activation_tables(nc.m.arch).items())
    setid = next(i for i, (_, fns) in enumerate(tabs)
                 if AF.Exp in fns and AF.Ln in fns)
    nc.scalar.add_instruction(mybir.InstLoadActFuncSet(
        name=nc.get_next_instruction_name(), ins=[], outs=[],
        act_func_set_id=setid))


def build_nc():
    nc = bacc.Bacc("TRN2", target_bir_lowering=False, debug=False)

    x_in = nc.dram_tensor("x", [IPC * N, D], F32, kind="ExternalInput")
    y_in = nc.dram_tensor("y", [IPC * N, D], F32, kind="ExternalInput")
    idt_in = nc.dram_tensor("idt", [P, P], BF16, kind="ExternalInput")
    negid_in = nc.dram_tensor("negid", [P, P], BF16, kind="ExternalInput")
    out_d = nc.dram_tensor("out", [P, 3 * IPC], F32, kind="ExternalOutput")

    with tile.TileContext(nc) as tc, ExitStack() as ctx:
        const = ctx.enter_context(tc.tile_pool(name="const", bufs=1))
        nat = ctx.enter_context(tc.tile_pool(name="nat", bufs=2))
        xbp = ctx.enter_context(tc.tile_pool(name="xbp", bufs=2))
        opT = ctx.enter_context(tc.tile_pool(name="opT", bufs=2))
        eep = ctx.enter_context(tc.tile_pool(name="eep", bufs=8))
        stat = ctx.enter_context(tc.tile_pool(name="stat", bufs=2))
        scr = ctx.enter_context(tc.tile_pool(name="scr", bufs=3))
        cssb = ctx.enter_context(tc.tile_pool(name="cssb", bufs=3))
        finp = ctx.enter_context(tc.tile_pool(name="finp", bufs=1))
        dram = ctx.enter_context(tc.tile_pool(name="dram", bufs=2, space="DRAM"))
        ps2 = ctx.enter_context(tc.tile_pool(name="ps2", bufs=2, space="PSUM"))
        ps1 = ctx.enter_context(tc.tile_pool(name="ps1", bufs=2, space="PSUM"))
        psc = ctx.enter_context(tc.tile_pool(name="psc", bufs=2, space="PSUM"))

        _pin_act_table(nc)

        # input loads ride the gpsimd software-DGE queue: cheap to issue and
        # on separate DMA rings, so they never head-of-line-block the
        # store->XBAR-transpose chain on the HWDGE queue.
        def load_nat(tname, it, src):
            # two half-loads so the ssq chain starts on the first half while
            # the second is still in flight
            t = nat.tile([P, NT * D], F32, tag=f"nat{tname}", name=f"nat{tname}{it}")
            H = NT // 2
            for h in range(2):
                nc.gpsimd.dma_start(
                    t[:, h * H * D:(h + 1) * H * D].rearrange(
                        "p (m d) -> p m d", m=H),
                    src[it * N + h * H * P:it * N + (h + 1) * H * P, :]
                    .rearrange("(m p) d -> p m d", p=P))
            return t

        nx0 = load_nat("x", 0, x_in)
        ny0 = load_nat("y", 0, y_in)

        idt = const.tile([P, P], BF16, tag="idt")
        nc.sync.dma_start(idt[:], idt_in[:])
        negid = const.tile([P, P], BF16, tag="negid")
        nc.sync.dma_start(negid[:], negid_in[:])
        ones_b = const.tile([P, 1], BF16, tag="ones_b")
        nc.vector.memset(ones_b[:], 1.0)
        zeros_b = const.tile([P, 1], BF16, tag="zeros_b")
        nc.vector.memset(zeros_b[:], 0.0)
        ones_f = const.tile([P, 1], F32, tag="ones_f")
        nc.vector.memset(ones_f[:], 1.0)
        ln10c = const.tile([P, 1], F32, tag="ln10c")
        nc.vector.memset(ln10c[:], LN10)

        # fin columns per item: [sum ln Tx, sum ln Ty, -2*pos_sum]
        fin = finp.tile([P, 3 * IPC], F32, tag="fin")

        def prep_norm(tname, it, nt_, act_ssq=False):
            """fused ssq per row block (DVE, or ScalarE Square when the DVE
            is the gating engine), then 10/||row|| on ScalarE. Processed in
            halves so scaling can begin before the second half of the input
            DMA has landed."""
            H = NT // 2
            ss = stat.tile([P, NT], F32, tag=f"ss{tname}", name=f"ss{tname}{it}")
            inv10 = stat.tile([P, NT], F32, tag=f"inv{tname}", name=f"inv{tname}{it}")
            for h in range(2):
                for mt in range(h * H, (h + 1) * H):
                    sq = scr.tile([P, D], BF16, tag="sq", name="sq")
                    if act_ssq:
                        nc.scalar.activation(
                            sq[:], nt_[:, mt * D:(mt + 1) * D], AF.Square,
                            accum_out=ss[:, mt:mt + 1])
                    else:
                        nc.vector.scalar_tensor_tensor(
                            sq[:], nt_[:, mt * D:(mt + 1) * D], 1.0,
                            nt_[:, mt * D:(mt + 1) * D], ALU.mult, ALU.mult,
                            accum_out=ss[:, mt:mt + 1])
                lns = scr.tile([P, NT], F32, tag="lns", name="lns")
                nc.scalar.activation(lns[:, h * H:(h + 1) * H],
                                     ss[:, h * H:(h + 1) * H], AF.Ln)
                nc.scalar.activation(inv10[:, h * H:(h + 1) * H],
                                     lns[:, h * H:(h + 1) * H], AF.Exp,
                                     scale=-0.5, bias=ln10c[:])
            return inv10

        def prep_scale(tname, it, nt_, inv10, pe_transpose=False, hsplit=False):
            """scale+cast rows to bf16, then build the d-major operands."""
            b = xbp.tile([P, NT * D], BF16, tag=f"{tname}b", name=f"{tname}b{it}")
            for mt in range(NT):
                nc.vector.tensor_scalar(
                    b[:, mt * D:(mt + 1) * D], nt_[:, mt * D:(mt + 1) * D],
                    inv10[:, mt:mt + 1], None, ALU.mult)
            # operands are built as half-tiles ts[k][h] = [128 (d-chunk k),
            # 512 (rows h*512..)]: every matmul slice in the kernel aligns
            # with the 512-wide PSUM bank halves, and the DMA-transpose path
            # can ship each row-half as soon as its scaling lands.
            ts = [[None, None] for _ in range(KC)]
            H = NT // 2
            if pe_transpose:
                # PE transposes pipeline with the per-block scaling and skip
                # the DRAM roundtrip; drain copies run on ScalarE (idle here)
                # to keep the DVE free for the other operand's prep chain.
                for k in range(KC):
                    tp = ps1.tile([P, N], BF16, tag="ps1", name="tp")
                    for mt in range(NT):
                        nc.tensor.transpose(
                            tp[:, mt * P:(mt + 1) * P],
                            b[:, mt * D + k * P:mt * D + (k + 1) * P], idt[:])
                    for h in range(2):
                        tT = opT.tile([P, HB], BF16, tag=f"{tname}T{k}{h}",
                                      name=f"{tname}T{k}{h}_{it}")
                        nc.scalar.copy(tT[:], tp[:, h * HB:(h + 1) * HB])
                        ts[k][h] = tT
            elif hsplit:
                # per-row-half store + transpose: the first half ships while
                # the second is still being scaled. Costs 2x the Sync issue
                # slots, so only used where the operand latency is critical.
                bd = dram.tile([N, D], BF16, tag=f"{tname}bd", name=f"{tname}bd{it}")
                bv = b[:].rearrange("p (m d) -> p m d", m=NT)
                for h in range(2):
                    nc.sync.dma_start(
                        bd[h * HB:(h + 1) * HB, :].rearrange(
                            "(m p) d -> p m d", p=P),
                        bv[:, h * H:(h + 1) * H, :])
                    for k in range(KC):
                        tT = opT.tile([P, HB], BF16, tag=f"{tname}T{k}{h}",
                                      name=f"{tname}T{k}{h}_{it}")
                        nc.sync.dma_start_transpose(
                            tT[:], bd[h * HB:(h + 1) * HB, k * P:(k + 1) * P])
                        ts[k][h] = tT
            else:
                bd = dram.tile([N, D], BF16, tag=f"{tname}bd", name=f"{tname}bd{it}")
                nc.sync.dma_start(
                    bd[:].rearrange("(m p) d -> p m d", p=P),
                    b[:].rearrange("p (m d) -> p m d", m=NT))
                for k in range(KC):
                    tT = opT.tile([P, N], BF16, tag=f"{tname}T{k}",
                                  name=f"{tname}T{k}_{it}")
                    nc.sync.dma_start_transpose(tT[:], bd[:, k * P:(k + 1) * P])
                    for h in range(2):
                        ts[k][h] = tT[:, h * HB:(h + 1) * HB]
            return b, ts

        def roundtrip(vtag, it, drains):
            """PSUM [1, n] colsum vectors -> SBUF staging -> DRAM -> [128, 8]."""
            sb = cssb.tile([1, N], F32, tag="cs_sb", name=f"sb_{vtag}{it}")
            bcs = dram.tile([NT, P], F32, tag="bcs", name=f"bcs_{vtag}{it}")
            for (dst0, dst1, src) in drains:
                nc.vector.tensor_copy(sb[:, dst0:dst1], src)
            nc.sync.dma_start(bcs[:], sb[:])
            csT = stat.tile([P, NT], F32, tag=f"csT{vtag}", name=f"csT{vtag}{it}")
            nc.sync.dma_start(csT[:], bcs.rearrange("j p -> p j"))
            return csT

        def roundtrip_pe(vtag, it, drains):
            """Like roundtrip(), but the [1,n] -> [128,8] layout flip runs as
            8 tiny K=1 PE transposes into PSUM: ~3us less latency than the
            DRAM bounce. Used for the last phase, where it's tail-exposed
            and the PE is idle."""
            sb = cssb.tile([1, N], F32, tag="cs_sb", name=f"sb_{vtag}{it}")
            for (dst0, dst1, src) in drains:
                nc.vector.tensor_copy(sb[:, dst0:dst1], src)
            csp = psc.tile([P, NT], F32, tag="cs", name=f"csp_{vtag}{it}")
            for j in range(NT):
                nc.tensor.transpose(csp[:, j:j + 1], sb[0:1, j * P:(j + 1) * P],
                                    ones_f[0:1, 0:1])
            return csp

        # sym-phase row-block order: alternate between the ps1 pool (mt>=4,
        # one bank) and ps2 (mt<4, two banks) so up to 4 row-blocks are in
        # flight and the PE stream stays dense (keeps the HAM clock warm).
        SYM_ORDER = [4, 0, 5, 1, 6, 2, 3, 7]
        # last contributing mt per colsum half, in emission order
        _contrib = {nh: [mt for mt in SYM_ORDER
                         if max(mt * P + P, nh * HB) < min((nh + 1) * HB, N)]
                    for nh in range(2)}

        def sym_phase(oT, sname, it, bridge, warm=0, tail=False):
            """Upper-triangle similarity phase: returns (rowsums, csT)."""
            rs = stat.tile([P, NT], F32, tag=f"rs{sname}", name=f"rs{sname}{it}")
            cs = [psc.tile([1, HB], F32, tag="cs", name=f"cs{sname}{nh}_{it}")
                  for nh in range(2)]
            # open each accumulation group with a full-region zeroing matmul
            # so later partial-region contributors see uniform has_written
            # state (also makes cs[0][:, 0:128] valid zeros for the drain).
            # `bridge` (the scaled natural tile) is ready well before the
            # DMA-transposed operands, so openers + warm-up matmuls run in
            # the store->transpose latency window and bring the PE clock to
            # 2.4 GHz before the first real matmul.
            for nh in range(2):
                nc.tensor.matmul(cs[nh][:], zeros_b[:], bridge[:, 0:HB],
                                 start=True, stop=False)
            for w in range(warm):
                nc.tensor.matmul(cs[w % 2][:], zeros_b[:],
                                 bridge[:, 0:HB], start=False, stop=False)
            for mt in SYM_ORDER:
                lo = mt * P
                if lo < HB:
                    ps = ps2.tile([P, N], F32, tag="ps2", name="ps_sym")
                    base = 0
                    chunks = [(lo, HB), (HB, N)]
                else:
                    ps = ps1.tile([P, HB], F32, tag="ps1", name="ps_sym1")
                    base = HB
                    chunks = [(lo, N)]
                for ci, (c0, c1) in enumerate(chunks):
                    ch = c0 // HB
                    for k in range(KC):
                        nc.tensor.matmul(
                            ps[:, c0 - base:c1 - base],
                            oT[k][mt // 4][:, (mt % 4) * P:(mt % 4 + 1) * P],
                            oT[k][ch][:, c0 - ch * HB:c1 - ch * HB],
                            start=(k == 0),
                            stop=(k == KC - 1 and ci > 0))
                # diag mask: add -1e5*I to [lo, lo+P) inside the group
                nc.tensor.matmul(
                    ps[:, lo - base:lo - base + P], idt[:], negid[:],
                    start=False, stop=True)
                ee = eep.tile([P, N], BF16, tag="ee", name="ee_sym")
                nc.scalar.activation(ee[:, lo:], ps[:, lo - base:],
                                     AF.Exp, accum_out=rs[:, mt:mt + 1])
                # strictly-upper colsums (lower-triangle rowsums by symmetry)
                for nh in range(2):
                    a = max(lo + P, nh * HB)
                    b = min((nh + 1) * HB, N)
                    if a >= b:
                        continue
                    nc.tensor.matmul(
                        cs[nh][:, a - nh * HB:b - nh * HB],
                        ones_b[:], ee[:, a:b],
                        start=False,
                        stop=(mt == _contrib[nh][-1]))
            return rs, cs

        def xy_phase(it, xT, yT):
            rs_xy = stat.tile([P, NT], F32, tag="rs_xy", name=f"rs_xy{it}")
            cs_xy = [psc.tile([1, HB], F32, tag="cs", name=f"cs_xy{nh}_{it}")
                     for nh in range(2)]
            for mt in range(NT):
                ps = ps2.tile([P, N], F32, tag="ps2", name="ps_xy")
                for nh in range(2):
                    for k in range(KC):
                        nc.tensor.matmul(
                            ps[:, nh * HB:(nh + 1) * HB],
                            xT[k][mt // 4][:, (mt % 4) * P:(mt % 4 + 1) * P],
                            yT[k][nh][:],
                            start=(k == 0), stop=(k == KC - 1))
                ee = eep.tile([P, N], BF16, tag="ee", name="ee_xy")
                nc.scalar.activation(ee[:], ps[:], AF.Exp,
                                     accum_out=rs_xy[:, mt:mt + 1])
                for nh in range(2):
                    nc.tensor.matmul(
                        cs_xy[nh][:], ones_b[:], ee[:, nh * HB:(nh + 1) * HB],
                        start=(mt == 0), stop=(mt == NT - 1))
            return rs_xy, cs_xy

        def pos_diag(it, xb, yb):
            pos = stat.tile([P, NT], F32, tag="pos", name=f"pos{it}")
            for mt in range(NT):
                pq = scr.tile([P, D], BF16, tag="pq", name="pq")
                nc.vector.scalar_tensor_tensor(
                    pq[:], xb[:, mt * D:(mt + 1) * D], 1.0,
                    yb[:, mt * D:(mt + 1) * D], ALU.mult, ALU.mult,
                    accum_out=pos[:, mt:mt + 1])
            return pos

        _D = lambda cs: [(0, HB, cs[0][:]), (HB, N, cs[1][:])]

        # ---- software-pipelined emission: each engine's queue is FIFO, so
        # emission order IS the schedule. Norm (ScalarE) ops for the next
        # operand are emitted before a phase's exp stream; item1's prep is
        # interleaved between item0's phases.
        invx0 = prep_norm("x", 0, nx0)
        xb0, xT0 = prep_scale("x", 0, nx0, invx0, pe_transpose=True)
        invy0 = prep_norm("y", 0, ny0)
        yb0, yT0 = prep_scale("y", 0, ny0, invy0, hsplit=True)

        rs_xx0, cs_xx0 = sym_phase(xT0, "xx", 0, xb0, warm=4)
        csT_xx0 = roundtrip("xx", 0, _D(cs_xx0))

        nx1 = load_nat("x", 1, x_in)
        ny1 = load_nat("y", 1, y_in)
        invx1 = prep_norm("x", 1, nx1)
        xb1, xT1 = prep_scale("x", 1, nx1, invx1)

        rs_xy0, cs_xy0 = xy_phase(0, xT0, yT0)

        invy1 = prep_norm("y", 1, ny1)
        yb1, yT1 = prep_scale("y", 1, ny1, invy1)
        csT_xy0 = roundtrip("xy", 0, _D(cs_xy0))

        def combine(it, rs_xx, csT_xx, rs_xy, csT_xy, rs_yy, csT_yy, pos):
            tx = stat.tile([P, NT], F32, tag="tx", name=f"tx{it}")
            nc.vector.tensor_add(tx[:], rs_xy[:], rs_xx[:])
            nc.vector.tensor_add(tx[:], tx[:], csT_xx[:])
            lnx = scr.tile([P, NT], F32, tag="lnx", name="lnx")
            nc.scalar.activation(lnx[:], tx[:], AF.Ln,
                                 accum_out=fin[:, 3 * it:3 * it + 1])

            ty = stat.tile([P, NT], F32, tag="ty", name=f"ty{it}")
            nc.vector.tensor_add(ty[:], rs_yy[:], csT_xy[:])
            nc.vector.tensor_add(ty[:], ty[:], csT_yy[:])
            lny = scr.tile([P, NT], F32, tag="lny", name="lny")
            nc.scalar.activation(lny[:], ty[:], AF.Ln,
                                 accum_out=fin[:, 3 * it + 1:3 * it + 2])

            posr = stat.tile([P, 1], F32, tag="posr", name=f"posr{it}")
            nc.vector.reduce_sum(posr[:], pos[:], axis=AX.X)
            nc.vector.tensor_scalar_mul(fin[:, 3 * it + 2:3 * it + 3], posr[:], -2.0)

        rs_yy0, cs_yy0 = sym_phase(yT0, "yy", 0, yb0)
        csT_yy0 = roundtrip("yy", 0, _D(cs_yy0))
        pos0 = pos_diag(0, xb0, yb0)

        rs_xx1, cs_xx1 = sym_phase(xT1, "xx", 1, xb1)
        csT_xx1 = roundtrip("xx", 1, _D(cs_xx1))
        combine(0, rs_xx0, csT_xx0, rs_xy0, csT_xy0, rs_yy0, csT_yy0, pos0)

        rs_xy1, cs_xy1 = xy_phase(1, xT1, yT1)
        csT_xy1 = roundtrip("xy", 1, _D(cs_xy1))

        rs_yy1, cs_yy1 = sym_phase(yT1, "yy", 1, yb1)
        csT_yy1 = roundtrip_pe("yy", 1, _D(cs_yy1))
        pos1 = pos_diag(1, xb1, yb1)
        combine(1, rs_xx1, csT_xx1, rs_xy1, csT_xy1, rs_yy1, csT_yy1, pos1)

        # ship the [128, 6] partials raw; the host does the final
        # reduction (drops the fin-matmul -> copy -> reduce tail chain)
        nc.sync.dma_start(out_d[:], fin[:])

    nc.compile()
    return nc


_CACHE = {}
TRACE = False
LAST_RESULTS = None


def _get_nc():
    if "nc" not in _CACHE:
        _CACHE["nc"] = build_nc()
    return _CACHE["nc"]


def make_in_maps(x, y):
    import ml_dtypes

    x = np.ascontiguousarray(np.asarray(x, dtype=np.float32))
    y = np.ascontiguousarray(np.asarray(y, dtype=np.float32))
    idt = np.eye(P, dtype=np.float32).astype(ml_dtypes.bfloat16)
    negid = (np.eye(P, dtype=np.float32) * NEG).astype(ml_dtypes.bfloat16)
    in_maps = []
    for c in range(NCORES):
        in_maps.append({
            "x": x[c * IPC:(c + 1) * IPC].reshape(IPC * N, D),
            "y": y[c * IPC:(c + 1) * IPC].reshape(IPC * N, D),
            "idt": idt,
            "negid": negid,
        })
    return in_maps


def kernel(x, y):
    global LAST_RESULTS
    nc = _get_nc()
    in_maps = make_in_maps(x, y)
    res = run_bass_kernel_spmd(nc, in_maps, list(range(NCORES)), trace=TRACE)
    LAST_RESULTS = res
    partials = np.array([np.asarray(r["out"], dtype=np.float64).sum()
                         for r in res.results])
    return np.float32(partials.sum() / (BS * 2 * N))


# revision 87
# speedup vs baseline: 1.0238x; 1.0031x over previous
"""Trainium2 Bass kernel for a SimCLR-style contrastive loss (v4).

Math (per batch item b, xn/yn L2-normalized rows, tau = 0.01):
  x-row i logits = {S_xy[i, :]} u {S_xx[i, j != i]}    (2n-1 values)
  y-row j logits = {S_xy[:, j]} u {S_yy[j, i != j]}
  loss = mean over bs*2n rows of (logsumexp(logits) - S_xy[diag])

Per-core structure (data-parallel over bs, 2 items/core). Key design
points, driven by perfetto traces of earlier versions:

  - ONE strided DMA per tensor loads [n, d] into natural SBUF layout.
  - ssq via fused DVE scalar_tensor_tensor (accum_out); 10/||row|| =
    exp(-.5*ln(ss)+ln10) on ScalarE.  An explicit InstLoadActFuncSet pins
    the natural_log_exp table set so the whole kernel does ONE activation
    table load (the default placement thrashed exp/ln sets 7 times).
  - rows scaled+cast to bf16 by DVE tensor_scalar (fp32 2x mode).
  - operand transposes via the DMA XBAR (store scaled bf16 to DRAM, read
    back with dma_start_transpose): zero PE/DVE cost.
  - phase order XX -> XY -> YY so the PE can start as soon as x alone is
    prepped (~20us earlier than an XY-first order).
  - S_xy: full matrix; rowsums from ACT exp accum_out, colsums via
    ones-vector PE matmuls accumulated in PSUM.
  - S_xx/S_yy: upper triangle only; diagonal masked by a -1e5*identity
    matmul folded into the PSUM accumulation; row totals = ACT rowsum of
    the [lo, n) strip + colsums of the strictly-upper blocks (symmetry),
    moved to [128, 8] layout via a [1,n] DRAM roundtrip per phase.
  - pos via fused DVE mul-reduce, emitted after the matmul phases (off
    the critical path); ln on ACT with accum; one ones-matmul collapses
    partitions; host sums the 8 per-core partials.
"""

from contextlib import ExitStack

import numpy as np

import concourse.bacc as bacc
import concourse.tile as tile
from concourse import mybir
from concourse.bass_utils import run_bass_kernel_spmd

BS, N, D = 16, 1024, 256
NCORES = 8
IPC = BS // NCORES  # items per core
P = 128
NT = N // P  # 128-row blocks per item
KC = D // P  # contraction chunks
HB = 512  # one PSUM bank of fp32
NEG = -100000.0  # folded into S_xx/S_yy diag -> exp() == 0.0
LN10 = 2.302585092994046

dt = mybir.dt
AF = mybir.ActivationFunctionType
ALU = mybir.AluOpType
AX = mybir.AxisListType
F32 = dt.float32
BF16 = dt.bfloat16


def _pin_act_table(nc):
    """Emit an explicit table load for the set containing BOTH Exp and Ln,
    so bacc's fixpoint pass never needs to swap tables mid-kernel."""
    from concourse.hw_specs import get_activation_tables

    tabs = list(get_activation_tables(nc.m.arch).items())
    setid = next(i for i, (_, fns) in enumerate(tabs)
                 if AF.Exp in fns and AF.Ln in fns)
    nc.scalar.add_instruction(mybir.InstLoadActFuncSet(
        name=nc.get_next_instruction_name(), ins=[], outs=[],
        act_func_set_id=setid))


def build_nc():
    nc = bacc.Bacc("TRN2", target_bir_lowering=False, debug=False)

    x_in = nc.dram_tensor("x", [IPC * N, D], F32, kind="ExternalInput")
    y_in = nc.dram_tensor("y", [IPC * N, D], F32, kind="ExternalInput")
    idt_in = nc.dram_tensor("idt", [P, P], BF16, kind="ExternalInput")
    negid_in = nc.dram_tensor("negid", [P, P], BF16, kind="ExternalInput")
    out_d = nc.dram_tensor("out", [P, 3 * IPC], F32, kind="ExternalOutput")

    with tile.TileContext(nc) as tc, ExitStack() as ctx:
        const = ctx.enter_context(tc.tile_pool(name="const", bufs=1))
        nat = ctx.enter_context(tc.tile_pool(name="nat", bufs=2))
        xbp = ctx.enter_context(tc.tile_pool(name="xbp", bufs=2))
        opT = ctx.enter_context(tc.tile_pool(name="opT", bufs=2))
        eep = ctx.enter_context(tc.tile_pool(name="eep", bufs=8))
        stat = ctx.enter_context(tc.tile_pool(name="stat", bufs=2))
        scr = ctx.enter_context(tc.tile_pool(name="scr", bufs=3))
        cssb = ctx.enter_context(tc.tile_pool(name="cssb", bufs=3))
        finp = ctx.enter_context(tc.tile_pool(name="finp", bufs=1))
        dram = ctx.enter_context(tc.tile_pool(name="dram", bufs=2, space="DRAM"))
        ps2 = ctx.enter_context(tc.tile_pool(name="ps2", bufs=2, space="PSUM"))
        ps1 = ctx.enter_context(tc.tile_pool(name="ps1", bufs=2, space="PSUM"))
        psc = ctx.enter_context(tc.tile_pool(name="psc", bufs=2, space="PSUM"))

        _pin_act_table(nc)

        # input loads ride the gpsimd software-DGE queue: cheap to issue and
        # on separate DMA rings, so they never head-of-line-block the
        # store->XBAR-transpose chain on the HWDGE queue.
        def load_nat(tname, it, src):
            # two half-loads so the ssq chain starts on the first half while
            # the second is still in flight
            t = nat.tile([P, NT * D], F32, tag=f"nat{tname}", name=f"nat{tname}{it}")
            H = NT // 2
            for h in range(2):
                nc.gpsimd.dma_start(
                    t[:, h * H * D:(h + 1) * H * D].rearrange(
                        "p (m d) -> p m d", m=H),
                    src[it * N + h * H * P:it * N + (h + 1) * H * P, :]
                    .rearrange("(m p) d -> p m d", p=P))
            return t

        nx0 = load_nat("x", 0, x_in)
        ny0 = load_nat("y", 0, y_in)

        idt = const.tile([P, P], BF16, tag="idt")
        nc.sync.dma_start(idt[:], idt_in[:])
        negid = const.tile([P, P], BF16, tag="negid")
        nc.sync.dma_start(negid[:], negid_in[:])
        ones_b = const.tile([P, 1], BF16, tag="ones_b")
        nc.vector.memset(ones_b[:], 1.0)
        zeros_b = const.tile([P, 1], BF16, tag="zeros_b")
        nc.vector.memset(zeros_b[:], 0.0)
        ones_f = const.tile([P, 1], F32, tag="ones_f")
        nc.vector.memset(ones_f[:], 1.0)
        ln10c = const.tile([P, 1], F32, tag="ln10c")
        nc.vector.memset(ln10c[:], LN10)

        # fin columns per item: [sum ln Tx, sum ln Ty, -2*pos_sum]
        fin = finp.tile([P, 3 * IPC], F32, tag="fin")

        # HAM warm-up: 3 fp32 N=512 matmuls (~1.7us each at the cold clock)
        # gated only on the first input half, filling the otherwise-idle PE
        # window at ~10-15us. PE transposes don't count as HAM activity, so
        # without this the transposes AND the first ~3.4us of real matmuls
        # all run at 1.2 GHz (throttle_active measured 14-16us per run).
        warm_ps = ps2.tile([1, HB], F32, tag="ps2", name="warm_ps")
        for w in range(3):
            nc.tensor.matmul(warm_ps[:], ones_f[:], nx0[:, 0:HB],
                             start=True, stop=True)

        def prep_norm(tname, it, nt_, act_ssq=False):
            """fused ssq per row block (DVE, or ScalarE Square when the DVE
            is the gating engine), then 10/||row|| on ScalarE. Processed in
            halves so scaling can begin before the second half of the input
            DMA has landed."""
            H = NT // 2
            ss = stat.tile([P, NT], F32, tag=f"ss{tname}", name=f"ss{tname}{it}")
            inv10 = stat.tile([P, NT], F32, tag=f"inv{tname}", name=f"inv{tname}{it}")
            for h in range(2):
                for mt in range(h * H, (h + 1) * H):
                    sq = scr.tile([P, D], BF16, tag="sq", name="sq")
                    if act_ssq:
                        nc.scalar.activation(
                            sq[:], nt_[:, mt * D:(mt + 1) * D], AF.Square,
                            accum_out=ss[:, mt:mt + 1])
                    else:
                        nc.vector.scalar_tensor_tensor(
                            sq[:], nt_[:, mt * D:(mt + 1) * D], 1.0,
                            nt_[:, mt * D:(mt + 1) * D], ALU.mult, ALU.mult,
                            accum_out=ss[:, mt:mt + 1])
                lns = scr.tile([P, NT], F32, tag="lns", name="lns")
                nc.scalar.activation(lns[:, h * H:(h + 1) * H],
                                     ss[:, h * H:(h + 1) * H], AF.Ln)
                nc.scalar.activation(inv10[:, h * H:(h + 1) * H],
                                     lns[:, h * H:(h + 1) * H], AF.Exp,
                                     scale=-0.5, bias=ln10c[:])
            return inv10

        def prep_scale(tname, it, nt_, inv10, pe_transpose=False, hsplit=False):
            """scale+cast rows to bf16, then build the d-major operands."""
            b = xbp.tile([P, NT * D], BF16, tag=f"{tname}b", name=f"{tname}b{it}")
            for mt in range(NT):
                nc.vector.tensor_scalar(
                    b[:, mt * D:(mt + 1) * D], nt_[:, mt * D:(mt + 1) * D],
                    inv10[:, mt:mt + 1], None, ALU.mult)
            # operands are built as half-tiles ts[k][h] = [128 (d-chunk k),
            # 512 (rows h*512..)]: every matmul slice in the kernel aligns
            # with the 512-wide PSUM bank halves, and the DMA-transpose path
            # can ship each row-half as soon as its scaling lands.
            ts = [[None, None] for _ in range(KC)]
            H = NT // 2
            if pe_transpose:
                # PE transposes pipeline with the per-block scaling and skip
                # the DRAM roundtrip; drain copies run on ScalarE (idle here)
                # to keep the DVE free for the other operand's prep chain.
                for k in range(KC):
                    tp = ps1.tile([P, N], BF16, tag="ps1", name="tp")
                    for mt in range(NT):
                        nc.tensor.transpose(
                            tp[:, mt * P:(mt + 1) * P],
                            b[:, mt * D + k * P:mt * D + (k + 1) * P], idt[:])
                    for h in range(2):
                        tT = opT.tile([P, HB], BF16, tag=f"{tname}T{k}{h}",
                                      name=f"{tname}T{k}{h}_{it}")
                        nc.scalar.copy(tT[:], tp[:, h * HB:(h + 1) * HB])
                        ts[k][h] = tT
            elif hsplit:
                # per-row-half store + transpose: the first half ships while
                # the second is still being scaled. Costs 2x the Sync issue
                # slots, so only used where the operand latency is critical.
                bd = dram.tile([N, D], BF16, tag=f"{tname}bd", name=f"{tname}bd{it}")
                bv = b[:].rearrange("p (m d) -> p m d", m=NT)
                for h in range(2):
                    nc.sync.dma_start(
                        bd[h * HB:(h + 1) * HB, :].rearrange(
                            "(m p) d -> p m d", p=P),
                        bv[:, h * H:(h + 1) * H, :])
                    for k in range(KC):
                        tT = opT.tile([P, HB], BF16, tag=f"{tname}T{k}{h}",
                                      name=f"{tname}T{k}{h}_{it}")
                        nc.sync.dma_start_transpose(
                            tT[:], bd[h * HB:(h + 1) * HB, k * P:(k + 1) * P])
                        ts[k][h] = tT
            else:
                bd = dram.tile([N, D], BF16, tag=f"{tname}bd", name=f"{tname}bd{it}")
                nc.sync.dma_start(
                    bd[:].rearrange("(m p) d -> p m d", p=P),
                    b[:].rearrange("p (m d) -> p m d", m=NT))
                for k in range(KC):
                    tT = opT.tile([P, N], BF16, tag=f"{tname}T{k}",
                                  name=f"{tname}T{k}_{it}")
                    nc.sync.dma_start_transpose(tT[:], bd[:, k * P:(k + 1) * P])
                    for h in range(2):
                        ts[k][h] = tT[:, h * HB:(h + 1) * HB]
            return b, ts

        def roundtrip(vtag, it, drains):
            """PSUM [1, n] colsum vectors -> SBUF staging -> DRAM -> [128, 8]."""
            sb = cssb.tile([1, N], F32, tag="cs_sb", name=f"sb_{vtag}{it}")
            bcs = dram.tile([NT, P], F32, tag="bcs", name=f"bcs_{vtag}{it}")
            for (dst0, dst1, src) in drains:
                nc.vector.tensor_copy(sb[:, dst0:dst1], src)
            nc.sync.dma_start(bcs[:], sb[:])
            csT = stat.tile([P, NT], F32, tag=f"csT{vtag}", name=f"csT{vtag}{it}")
            nc.sync.dma_start(csT[:], bcs.rearrange("j p -> p j"))
            return csT

        def roundtrip_pe(vtag, it, drains):
            """Like roundtrip(), but the [1,n] -> [128,8] layout flip runs as
            8 tiny K=1 PE transposes into PSUM: ~3us less latency than the
            DRAM bounce. Used for the last phase, where it's tail-exposed
            and the PE is idle."""
            sb = cssb.tile([1, N], F32, tag="cs_sb", name=f"sb_{vtag}{it}")
            for (dst0, dst1, src) in drains:
                nc.vector.tensor_copy(sb[:, dst0:dst1], src)
            csp = psc.tile([P, NT], F32, tag="cs", name=f"csp_{vtag}{it}")
            for j in range(NT):
                nc.tensor.transpose(csp[:, j:j + 1], sb[0:1, j * P:(j + 1) * P],
                                    ones_f[0:1, 0:1])
            return csp

        # sym-phase row-block order: alternate between the ps1 pool (mt>=4,
        # one bank) and ps2 (mt<4, two banks) so up to 4 row-blocks are in
        # flight and the PE stream stays dense (keeps the HAM clock warm).
        SYM_ORDER = [4, 0, 5, 1, 6, 2, 3, 7]
        # last contributing mt per colsum half, in emission order
        _contrib = {nh: [mt for mt in SYM_ORDER
                         if max(mt * P + P, nh * HB) < min((nh + 1) * HB, N)]
                    for nh in range(2)}

        def sym_phase(oT, sname, it, bridge, warm=0, tail=False):
            """Upper-triangle similarity phase: returns (rowsums, csT)."""
            rs = stat.tile([P, NT], F32, tag=f"rs{sname}", name=f"rs{sname}{it}")
            cs = [psc.tile([1, HB], F32, tag="cs", name=f"cs{sname}{nh}_{it}")
                  for nh in range(2)]
            # open each accumulation group with a full-region zeroing matmul
            # so later partial-region contributors see uniform has_written
            # state (also makes cs[0][:, 0:128] valid zeros for the drain).
            # `bridge` (the scaled natural tile) is ready well before the
            # DMA-transposed operands, so openers + warm-up matmuls run in
            # the store->transpose latency window and bring the PE clock to
            # 2.4 GHz before the first real matmul.
            for nh in range(2):
                nc.tensor.matmul(cs[nh][:], zeros_b[:], bridge[:, 0:HB],
                                 start=True, stop=False)
            for w in range(warm):
                nc.tensor.matmul(cs[w % 2][:], zeros_b[:],
                                 bridge[:, 0:HB], start=False, stop=False)
            for mt in SYM_ORDER:
                lo = mt * P
                if lo < HB:
                    ps = ps2.tile([P, N], F32, tag="ps2", name="ps_sym")
                    base = 0
                    chunks = [(lo, HB), (HB, N)]
                else:
                    ps = ps1.tile([P, HB], F32, tag="ps1", name="ps_sym1")
                    base = HB
                    chunks = [(lo, N)]
                for ci, (c0, c1) in enumerate(chunks):
                    ch = c0 // HB
                    for k in range(KC):
                        nc.tensor.matmul(
                            ps[:, c0 - base:c1 - base],
                            oT[k][mt // 4][:, (mt % 4) * P:(mt % 4 + 1) * P],
                            oT[k][ch][:, c0 - ch * HB:c1 - ch * HB],
                            start=(k == 0),
                            stop=(k == KC - 1 and ci > 0))
                # diag mask: add -1e5*I to [lo, lo+P) inside the group
                nc.tensor.matmul(
                    ps[:, lo - base:lo - base + P], idt[:], negid[:],
                    start=False, stop=True)
                ee = eep.tile([P, N], BF16, tag="ee", name="ee_sym")
                nc.scalar.activation(ee[:, lo:], ps[:, lo - base:],
                                     AF.Exp, accum_out=rs[:, mt:mt + 1])
                # strictly-upper colsums (lower-triangle rowsums by symmetry)
                for nh in range(2):
                    a = max(lo + P, nh * HB)
                    b = min((nh + 1) * HB, N)
                    if a >= b:
                        continue
                    nc.tensor.matmul(
                        cs[nh][:, a - nh * HB:b - nh * HB],
                        ones_b[:], ee[:, a:b],
                        start=False,
                        stop=(mt == _contrib[nh][-1]))
            return rs, cs

        def xy_phase(it, xT, yT):
            rs_xy = stat.tile([P, NT], F32, tag="rs_xy", name=f"rs_xy{it}")
            cs_xy = [psc.tile([1, HB], F32, tag="cs", name=f"cs_xy{nh}_{it}")
                     for nh in range(2)]
            for mt in range(NT):
                ps = ps2.tile([P, N], F32, tag="ps2", name="ps_xy")
                for nh in range(2):
                    for k in range(KC):
                        nc.tensor.matmul(
                            ps[:, nh * HB:(nh + 1) * HB],
                            xT[k][mt // 4][:, (mt % 4) * P:(mt % 4 + 1) * P],
                            yT[k][nh][:],
                            start=(k == 0), stop=(k == KC - 1))
                ee = eep.tile([P, N], BF16, tag="ee", name="ee_xy")
                nc.scalar.activation(ee[:], ps[:], AF.Exp,
                                     accum_out=rs_xy[:, mt:mt + 1])
                for nh in range(2):
                    nc.tensor.matmul(
                        cs_xy[nh][:], ones_b[:], ee[:, nh * HB:(nh + 1) * HB],
                        start=(mt == 0), stop=(mt == NT - 1))
            return rs_xy, cs_xy

        def pos_diag(it, xb, yb):
            pos = stat.tile([P, NT], F32, tag="pos", name=f"pos{it}")
            for mt in range(NT):
                pq = scr.tile([P, D], BF16, tag="pq", name="pq")
                nc.vector.scalar_tensor_tensor(
                    pq[:], xb[:, mt * D:(mt + 1) * D], 1.0,
                    yb[:, mt * D:(mt + 1) * D], ALU.mult, ALU.mult,
                    accum_out=pos[:, mt:mt + 1])
            return pos

        _D = lambda cs: [(0, HB, cs[0][:]), (HB, N, cs[1][:])]

        # ---- software-pipelined emission: each engine's queue is FIFO, so
        # emission order IS the schedule. Norm (ScalarE) ops for the next
        # operand are emitted before a phase's exp stream; item1's prep is
        # interleaved between item0's phases.
        invx0 = prep_norm("x", 0, nx0)
        xb0, xT0 = prep_scale("x", 0, nx0, invx0, pe_transpose=True)
        invy0 = prep_norm("y", 0, ny0)
        yb0, yT0 = prep_scale("y", 0, ny0, invy0, hsplit=True)

        rs_xx0, cs_xx0 = sym_phase(xT0, "xx", 0, xb0, warm=4)
        csT_xx0 = roundtrip("xx", 0, _D(cs_xx0))

        nx1 = load_nat("x", 1, x_in)
        ny1 = load_nat("y", 1, y_in)
        invx1 = prep_norm("x", 1, nx1)
        xb1, xT1 = prep_scale("x", 1, nx1, invx1)

        rs_xy0, cs_xy0 = xy_phase(0, xT0, yT0)

        invy1 = prep_norm("y", 1, ny1)
        yb1, yT1 = prep_scale("y", 1, ny1, invy1)
        csT_xy0 = roundtrip("xy", 0, _D(cs_xy0))

        def combine(it, rs_xx, csT_xx, rs_xy, csT_xy, rs_yy, csT_yy, pos):
            tx = stat.tile([P, NT], F32, tag="tx", name=f"tx{it}")
            nc.vector.tensor_add(tx[:], rs_xy[:], rs_xx[:])
            nc.vector.tensor_add(tx[:], tx[:], csT_xx[:])
            lnx = scr.tile([P, NT], F32, tag="lnx", name="lnx")
            nc.scalar.activation(lnx[:], tx[:], AF.Ln,
                                 accum_out=fin[:, 3 * it:3 * it + 1])

            ty = stat.tile([P, NT], F32, tag="ty", name=f"ty{it}")
            nc.vector.tensor_add(ty[:], rs_yy[:], csT_xy[:])
            nc.vector.tensor_add(ty[:], ty[:], csT_yy[:])
            lny = scr.tile([P, NT], F32, tag="lny", name="lny")
            nc.scalar.activation(lny[:], ty[:], AF.Ln,
                                 accum_out=fin[:, 3 * it + 1:3 * it + 2])

            posr = stat.tile([P, 1], F32, tag="posr", name=f"posr{it}")
            nc.vector.reduce_sum(posr[:], pos[:], axis=AX.X)
            nc.vector.tensor_scalar_mul(fin[:, 3 * it + 2:3 * it + 3], posr[:], -2.0)

        rs_yy0, cs_yy0 = sym_phase(yT0, "yy", 0, yb0)
        csT_yy0 = roundtrip("yy", 0, _D(cs_yy0))
        pos0 = pos_diag(0, xb0, yb0)

        rs_xx1, cs_xx1 = sym_phase(xT1, "xx", 1, xb1)
        csT_xx1 = roundtrip("xx", 1, _D(cs_xx1))
        combine(0, rs_xx0, csT_xx0, rs_xy0, csT_xy0, rs_yy0, csT_yy0, pos0)

        rs_xy1, cs_xy1 = xy_phase(1, xT1, yT1)
        csT_xy1 = roundtrip("xy", 1, _D(cs_xy1))

        rs_yy1, cs_yy1 = sym_phase(yT1, "yy", 1, yb1)
        csT_yy1 = roundtrip_pe("yy", 1, _D(cs_yy1))
        pos1 = pos_diag(1, xb1, yb1)
        combine(1, rs_xx1, csT_xx1, rs_xy1, csT_xy1, rs_yy1, csT_yy1, pos1)

        # ship the [128, 6] partials raw; the host does the final
        # reduction (drops the fin-matmul -> copy -> reduce tail chain)
        nc.sync.dma_start(out_d[:], fin[:])

    nc.compile()
    return nc


_CACHE = {}
TRACE = False
LAST_RESULTS = None


def _get_nc():
    if "nc" not in _CACHE:
        _CACHE["nc"] = build_nc()
    return _CACHE["nc"]


def make_in_maps(x, y):
    import ml_dtypes

    x = np.ascontiguousarray(np.asarray(x, dtype=np.float32))
    y = np.ascontiguousarray(np.asarray(y, dtype=np.float32))
    idt = np.eye(P, dtype=np.float32).astype(ml_dtypes.bfloat16)
    negid = (np.eye(P, dtype=np.float32) * NEG).astype(ml_dtypes.bfloat16)
    in_maps = []
    for c in range(NCORES):
        in_maps.append({
            "x": x[c * IPC:(c + 1) * IPC].reshape(IPC * N, D),
            "y": y[c * IPC:(c + 1) * IPC].reshape(IPC * N, D),
            "idt": idt,
            "negid": negid,
        })
    return in_maps


def kernel(x, y):
    global LAST_RESULTS
    nc = _get_nc()
    in_maps = make_in_maps(x, y)
    res = run_bass_kernel_spmd(nc, in_maps, list(range(NCORES)), trace=TRACE)
    LAST_RESULTS = res
    partials = np.array([np.asarray(r["out"], dtype=np.float64).sum()
                         for r in res.results])
    return np.float32(partials.sum() / (BS * 2 * N))
